# revision 15
# baseline (speedup 1.0000x reference)
"""Trainium2 Bass kernel for nn_ConvAlignLoss (8-core data parallel), v2.

Self-contained: hardcodes shapes; imports concourse from /opt/trn_rl_repo.

Per core (R=64 rows):
  loss_astf partial: sum((pred-true)^2)   (folded into the 16K block loop)
  conv = irfft16384(fft(pred) * conj(fft(egf_pad)))[:14337]  (2-stage matmul FFT)
  cc   = irfft32768(fft(conv_pad) * conj(fft(target_pad)))
  shift = argmax of cc == argmin of shift-encoded value at row max
  loss_conv partial: sum((conv[(7040+i+shift) % 14337] - target[7040+i])^2)
Host combines the 8 cores' (sum_astf, sum_conv) into the scalar losses.

v2 structural changes vs v1 (713us):
  - stage-1 matmuls emit (re|im) in one pass via stacked rhs [W1r|W1i]
  - inverse G-stage emits (Gr|Gi) in one psum via stacked rhs [V2r|V2i]
  - twiddle cmuls batched to full-block strided TTs (bf16 2x_1p mode)
  - input casts moved off Pool onto ACT
  - per-(t1,row) argmax candidates computed inline in the 32K loop; the
    global argmax tail is tiny
  - scratch writes issued from Pool so the 32K loads (sync) prefetch early
  - consts packed into 4 stacked dram tensors (few DMAs, sliced in SBUF)
"""
import sys

sys.path.insert(0, "/opt/trn_rl_repo")

import numpy as np
import concourse.bass as bass
import concourse.bacc as bacc
import concourse.mybir as mybir
from concourse import tile

F32 = mybir.dt.float32
BF16 = mybir.dt.bfloat16
I32 = mybir.dt.int32
AT = mybir.AluOpType
AX = mybir.AxisListType

R = 64
NCORES = 8
L1, L2 = 16384, 2048
CONV_LEN = L1 - L2 + 1      # 14337
N_A, N_B = 16384, 32768
GAP_LO, GAP_HI = CONV_LEN, N_B - CONV_LEN + 1   # gap [14337, 18432)
CROP = 256
START0 = (CONV_LEN - CROP) // 2                 # 7040
PITCH = 14720
BIGL = float(2 ** 23)
F1 = 65                      # Hermitian half: f1 in [0, 64]


def _dft(n, sign):
    k = np.arange(n)
    return np.exp(sign * 2j * np.pi * np.outer(k, k) / n)


def make_packed_consts():
    """Build the packed const arrays + per-name (group, offset, width)."""
    cf = np.ones(F1)
    cf[1:64] = 2.0

    W1 = _dft(128, -1)
    V2A = _dft(128, +1)
    twA = np.exp(-2j * np.pi * np.outer(np.arange(128), np.arange(F1)) / N_A)
    itwA = np.exp(2j * np.pi * np.outer(np.arange(F1), np.arange(128)) / N_A)
    V1A = cf[:, None] * _dft(128, +1)[:F1] / N_A        # [f1<=64, t1]

    W2B = _dft(256, -1)          # [t2, f2]
    twB = np.exp(-2j * np.pi * np.outer(np.arange(256), np.arange(F1)) / N_B)
    V2B = _dft(256, +1)          # [f2, t2]
    itwB = np.exp(2j * np.pi * np.outer(np.arange(F1), np.arange(256)) / N_B)
    V1B = cf[:, None] * _dft(128, +1)[:F1] / N_B        # [f1<=64, t1]

    # twB tiled: [128, (c,q,f)=520]; same tw for every q
    twBtr = np.hstack([np.tile(twB.real[c * 128:(c + 1) * 128], (1, 4))
                       for c in range(2)])
    twBti = np.hstack([np.tile(twB.imag[c * 128:(c + 1) * 128], (1, 4))
                       for c in range(2)])

    j = np.arange(128)[:, None] * 256 + np.arange(256)[None, :]   # [t1, t2]
    gap = (j >= GAP_LO) & (j < GAP_HI)
    maskB = np.where(gap, -1e30, 0.0)
    shiftval = np.where(j <= CONV_LEN - 1, j - (CONV_LEN - 1), j - GAP_HI + 1)
    shvB = np.where(gap, 0.0, shiftval - BIGL)
    winidx = (np.arange(R)[:, None] * PITCH
              + np.arange(CROP)[None, :])                         # [64, 256]

    groups = {}   # gname -> list of (name, arr)

    def put(g, name, arr):
        groups.setdefault(g, []).append(
            (name, np.ascontiguousarray(np.asarray(arr, np.float64))))

    # ---- [128, X] bf16 group ----
    put("cb128", "W1ri65", np.hstack([W1.real[:, :F1], W1.imag[:, :F1]]))
    put("cb128", "W1r", W1.real)      # row 0 is all-ones (used as ones-row)
    put("cb128", "nW1i", -W1.imag)
    put("cb128", "W1i", W1.imag)
    put("cb128", "twAr", np.tile(twA.real, (1, 8)))
    put("cb128", "twAi", np.tile(twA.imag, (1, 8)))
    put("cb128", "V2A_rI", np.hstack([V2A.real, V2A.imag]))
    put("cb128", "V2A_iR", np.hstack([-V2A.imag, V2A.real]))
    for a in range(2):
        for b in range(2):
            blk = W2B[a * 128:(a + 1) * 128, b * 128:(b + 1) * 128]
            put("cb128", f"W2Br{a}{b}", blk.real)
            put("cb128", f"W2Bi{a}{b}", blk.imag)
            put("cb128", f"nW2Bi{a}{b}", -blk.imag)
    put("cb128", "twBtr", twBtr)
    put("cb128", "twBti", twBti)
    for a in range(2):
        blk = V2B[a * 128:(a + 1) * 128, :]
        put("cb128", f"V2B_rI{a}", np.hstack([blk.real, blk.imag]))
        put("cb128", f"V2B_iR{a}", np.hstack([-blk.imag, blk.real]))
    put("cb128", "ident", np.eye(128))
    # ---- [65, X] bf16 group ----
    put("cb65", "itwAr", np.tile(itwA.real, (1, 8)))
    put("cb65", "itwAi", np.tile(itwA.imag, (1, 8)))
    put("cb65", "V1Ar", V1A.real)
    put("cb65", "nV1Ai", -V1A.imag)
    put("cb65", "itwBr", np.tile(itwB.real, (1, 4)))
    put("cb65", "itwBi", np.tile(itwB.imag, (1, 4)))
    put("cb65", "V1Br", V1B.real)
    put("cb65", "nV1Bi", -V1B.imag)
    # ---- [128, X] f32 group ----
    put("cf128", "maskB", maskB)
    put("cf128", "shvB", shvB)
    put("cf128", "identF", np.eye(128))
    put("cf128", "ones128", np.ones((128, 1)))
    # ---- [64, X] f32 group ----
    put("cf64", "winidx", winidx)
    put("cf64", "ones64", np.ones((R, 1)))

    gdtype = {"cb128": "bf16", "cb65": "bf16", "cf128": "f32", "cf64": "f32"}
    packed = {}
    layout = {}
    for g, items in groups.items():
        parts = []
        off = 0
        for name, arr in items:
            assert arr.ndim == 2
            layout[name] = (g, off, arr.shape[1])
            parts.append(arr)
            off += arr.shape[1]
        cat = np.hstack(parts)
        if gdtype[g] == "f32":
            packed[g] = cat.astype(np.float32)
        else:
            import ml_dtypes
            packed[g] = cat.astype(ml_dtypes.bfloat16)
    return packed, layout


def build_nc():
    nc = bacc.Bacc("TRN2", target_bir_lowering=False, debug=False,
                   num_devices=NCORES)
    packed, layout = make_packed_consts()

    pred = nc.dram_tensor("pred", [R, L1], F32, kind="ExternalInput")
    true_ = nc.dram_tensor("true", [R, L1], F32, kind="ExternalInput")
    egf = nc.dram_tensor("egf", [R, L2], F32, kind="ExternalInput")
    target = nc.dram_tensor("target", [R, CONV_LEN], F32,
                            kind="ExternalInput")
    out = nc.dram_tensor("out", [1, 2], F32, kind="ExternalOutput")
    scratch = nc.dram_tensor("scratch", [R, PITCH], BF16)

    cdram = {}
    for g, arr in packed.items():
        dt = F32 if arr.dtype == np.float32 else BF16
        cdram[g] = nc.dram_tensor(g, list(arr.shape), dt, kind="ExternalInput")

    NB1 = R // 8    # 8 blocks of 8 rows (16K level)
    NB2 = R // 4    # 16 blocks of 4 rows (32K level)

    def _cp(eng, out_ap, in_ap):
        if eng is nc.scalar:
            nc.scalar.copy(out_ap, in_ap)
        else:
            eng.tensor_copy(out_ap, in_ap)

    with tile.TileContext(nc) as tc:
        with (
            tc.tile_pool(name="consts", bufs=1) as cpool,
            tc.tile_pool(name="keep", bufs=1) as kpool,
        ):
            ctile = {}
            for g, arr in packed.items():
                dt = F32 if arr.dtype == np.float32 else BF16
                t = cpool.tile(list(arr.shape), dt, tag=f"c_{g}", name=f"c_{g}")
                nc.sync.dma_start(t[:], cdram[g][:])
                ctile[g] = t

            def cs(name):
                g, off, w = layout[name]
                return ctile[g][:, off:off + w]

            astf_acc = kpool.tile([128, NB1], F32, tag="astfacc", name="astfacc")
            allmax = kpool.tile([128, R], BF16, tag="allmax", name="allmax")
            argshv = kpool.tile([128, R], F32, tag="argshv", name="argshv")
            shifts = kpool.tile([R, 1], F32, tag="shifts", name="shifts")
            outt = kpool.tile([1, 2], F32, tag="outt", name="outt")

            # egf loaded once: [16, R*128] f32 -> bf16
            egf_f = kpool.tile([16, R * 128], F32, tag="egff", name="egff")
            egf_b = kpool.tile([16, R * 128], BF16, tag="egfb", name="egfb")
            nc.sync.dma_start(
                egf_f[:].rearrange("p (r b) -> p r b", b=128),
                egf.ap().rearrange("r (a b) -> a r b", a=16))
            nc.scalar.copy(egf_b[:], egf_f[:])

            # PSUM pools: 6 banks double-buffered + 2 single (8 total)
            _ps2ctx = tc.tile_pool(name="ps2", bufs=2, space="PSUM")
            pp = _ps2ctx.__enter__()
            _ps1ctx = tc.tile_pool(name="ps1", bufs=1, space="PSUM")
            pq = _ps1ctx.__enter__()

            # ---------------- B) 16K level + astf ----------------
            with tc.tile_pool(name="p16", bufs=2) as dp:
                for b in range(NB1):
                    r0 = b * 8
                    Dp = dp.tile([128, 1024], F32, tag="Dp", name="Dp")
                    Dt = dp.tile([128, 1024], F32, tag="Dt", name="Dt")
                    Dpb = dp.tile([128, 1024], BF16, tag="Dpb", name="Dpb")
                    psrc = pred[r0:r0 + 8, :].rearrange(
                        "q (a b) -> q a b", a=128).transpose([1, 0, 2])
                    tsrc = true_[r0:r0 + 8, :].rearrange(
                        "q (a b) -> q a b", a=128).transpose([1, 0, 2])
                    nc.sync.dma_start(
                        Dp[:].rearrange("p (q b) -> p q b", b=128), psrc)
                    nc.sync.dma_start(
                        Dt[:].rearrange("p (q b) -> p q b", b=128), tsrc)
                    # astf: Dt = Dp - Dt (Pool); accum (Dt*Dt) (DVE)
                    nc.gpsimd.tensor_tensor(Dt[:], Dp[:], Dt[:], op=AT.subtract)
                    nc.vector.scalar_tensor_tensor(
                        Dt[:], Dt[:], 1.0, Dt[:], op0=AT.bypass, op1=AT.mult,
                        accum_out=astf_acc[:, b:b + 1])
                    nc.scalar.copy(Dpb[:], Dp[:])   # cast f32->bf16 (ACT)

                    # --- s1: A[t2, (inp,ri,q,f1)], ri-major ---
                    A = dp.tile([128, 2080], BF16, tag="A16", name="A16")
                    cp_eng = [nc.scalar, nc.scalar, nc.vector, nc.scalar,
                              nc.scalar, nc.vector, nc.scalar, nc.scalar]
                    for qp in range(4):
                        ps = pp.tile([128, 260], F32, tag="s1", name="s1")
                        for h in range(2):
                            q = qp * 2 + h
                            nc.tensor.matmul(
                                ps[:, h * 130:(h + 1) * 130],
                                lhsT=Dpb[:, q * 128:(q + 1) * 128],
                                rhs=cs("W1ri65"), start=True, stop=True)
                        dst4 = A[:, 0:1040].rearrange(
                            "p (r q f) -> p r q f", r=2, q=8)[
                            :, :, 2 * qp:2 * qp + 2, :]
                        _cp(cp_eng[qp], dst4,
                            ps[:].rearrange("p (h r f) -> p r h f", h=2, r=2))
                    for qp in range(4):
                        ps = pp.tile([128, 260], F32, tag="s1", name="s1")
                        for h in range(2):
                            q = qp * 2 + h
                            gq = r0 + q
                            nc.tensor.matmul(
                                ps[:, h * 130:(h + 1) * 130],
                                lhsT=egf_b[:, gq * 128:(gq + 1) * 128],
                                rhs=cs("W1ri65")[0:16, :], start=True, stop=True)
                        dst4 = A[:, 1040:2080].rearrange(
                            "p (r q f) -> p r q f", r=2, q=8)[
                            :, :, 2 * qp:2 * qp + 2, :]
                        _cp(cp_eng[4 + qp], dst4,
                            ps[:].rearrange("p (h r f) -> p r h f", h=2, r=2))

                    # --- fwd twiddle cmul: B = A * twA (contig 520 runs) ---
                    B = dp.tile([128, 2080], BF16, tag="B16", name="B16")
                    tmpV = dp.tile([128, 1040], BF16, tag="tmpV", name="tmpV")
                    tmpP = dp.tile([128, 1040], BF16, tag="tmpP", name="tmpP")

                    def v16(t, lo, hi):
                        return t[:].rearrange("p (i x) -> p i x", i=2)[:, :, lo:hi]

                    twr = cs("twAr").rearrange("p (a x) -> p a x", a=1)\
                        .to_broadcast([128, 2, 520])
                    twi = cs("twAi").rearrange("p (a x) -> p a x", a=1)\
                        .to_broadcast([128, 2, 520])
                    ar, ai = v16(A, 0, 520), v16(A, 520, 1040)
                    br, bi = v16(B, 0, 520), v16(B, 520, 1040)
                    tv = tmpV[:].rearrange("p (i x) -> p i x", i=2)
                    tp = tmpP[:].rearrange("p (i x) -> p i x", i=2)
                    nc.vector.tensor_tensor(br, ar, twr, op=AT.mult)
                    nc.vector.tensor_tensor(tv, ai, twi, op=AT.mult)
                    nc.vector.tensor_tensor(br, br, tv, op=AT.subtract)
                    nc.gpsimd.tensor_tensor(bi, ar, twi, op=AT.mult)
                    nc.vector.tensor_tensor(tp, ai, twr, op=AT.mult)
                    nc.vector.tensor_tensor(bi, bi, tp, op=AT.add)

                    # --- s2: Z[f2, (inp,ri,g,260)] ---
                    Z = dp.tile([128, 2080], BF16, tag="Z16", name="Z16")
                    zc_eng = [nc.scalar, nc.vector, nc.scalar, nc.vector,
                              nc.scalar, nc.scalar, nc.scalar, nc.scalar]
                    zi_ = 0
                    for inp in range(2):
                        for g in range(2):
                            base = inp * 1040 + g * 260
                            rhs_r = B[:, base:base + 260]
                            rhs_i = B[:, base + 520:base + 780]
                            pzr = pq.tile([128, 260], F32, tag="s2r", name="s2r")
                            pzi = pq.tile([128, 260], F32, tag="s2i", name="s2i")
                            nc.tensor.matmul(pzr[:], lhsT=cs("W1r"),
                                             rhs=rhs_r,
                                             start=True, stop=False)
                            nc.tensor.matmul(pzr[:], lhsT=cs("nW1i"),
                                             rhs=rhs_i,
                                             start=False, stop=True)
                            nc.tensor.matmul(pzi[:], lhsT=cs("W1i"),
                                             rhs=rhs_r,
                                             start=True, stop=False)
                            nc.tensor.matmul(pzi[:], lhsT=cs("W1r"),
                                             rhs=rhs_i,
                                             start=False, stop=True)
                            _cp(zc_eng[zi_],
                                Z[:, inp * 1040 + g * 260:
                                  inp * 1040 + g * 260 + 260], pzr[:])
                            _cp(zc_eng[zi_ + 1],
                                Z[:, inp * 1040 + 520 + g * 260:
                                  inp * 1040 + 520 + g * 260 + 260], pzi[:])
                            zi_ += 2

                    # --- spectral: S = Zp * conj(Ze), contiguous 520 ---
                    S = dp.tile([128, 1040], BF16, tag="S16", name="S16")
                    zpr, zpi = Z[:, 0:520], Z[:, 520:1040]
                    zer, zei = Z[:, 1040:1560], Z[:, 1560:2080]
                    sr, si = S[:, 0:520], S[:, 520:1040]
                    tv2 = tmpV[:, 0:520]
                    tp2 = tmpP[:, 0:520]
                    nc.vector.tensor_tensor(sr, zpr, zer, op=AT.mult)
                    nc.vector.tensor_tensor(tv2, zpi, zei, op=AT.mult)
                    nc.vector.tensor_tensor(sr, sr, tv2, op=AT.add)
                    nc.gpsimd.tensor_tensor(si, zpi, zer, op=AT.mult)
                    nc.vector.tensor_tensor(tp2, zpr, zei, op=AT.mult)
                    nc.vector.tensor_tensor(si, si, tp2, op=AT.subtract)

                    # --- inverse G: psum (Gr|Gi) per q-pair ---
                    Gsb = dp.tile([F1, 2048], BF16, tag="G16", name="G16")
                    gc_eng = [nc.scalar, nc.vector, nc.scalar, nc.scalar]
                    for qp in range(4):
                        psG = pp.tile([F1, 512], F32, tag="G", name="G")
                        for h in range(2):
                            q = qp * 2 + h
                            g, qq = q // 4, q % 4
                            sro = g * 260 + qq * 65
                            sio = 520 + g * 260 + qq * 65
                            nc.tensor.matmul(
                                psG[:, h * 256:(h + 1) * 256],
                                lhsT=S[:, sro:sro + 65], rhs=cs("V2A_rI"),
                                start=True, stop=False)
                            nc.tensor.matmul(
                                psG[:, h * 256:(h + 1) * 256],
                                lhsT=S[:, sio:sio + 65], rhs=cs("V2A_iR"),
                                start=False, stop=True)
                        dstg = Gsb[:, 0:2048].rearrange(
                            "p (r q t) -> p r q t", r=2, q=8)[
                            :, :, 2 * qp:2 * qp + 2, :]
                        _cp(gc_eng[qp], dstg,
                            psG[:].rearrange("p (h r t) -> p r h t", h=2, r=2))

                    # --- itw cmul: H = G * itwA (contiguous 1024) ---
                    H = dp.tile([F1, 2048], BF16, tag="H16", name="H16")
                    itr = cs("itwAr")[0:F1, :]
                    iti = cs("itwAi")[0:F1, :]
                    gr, gi = Gsb[:, 0:1024], Gsb[:, 1024:2048]
                    hr, hi = H[:, 0:1024], H[:, 1024:2048]
                    tvh = tmpV[:F1, 0:1024]
                    tph = tmpP[:F1, 0:1024]
                    nc.vector.tensor_tensor(hr, gr, itr, op=AT.mult)
                    nc.vector.tensor_tensor(tvh, gi, iti, op=AT.mult)
                    nc.vector.tensor_tensor(hr, hr, tvh, op=AT.subtract)
                    nc.gpsimd.tensor_tensor(hi, gr, iti, op=AT.mult)
                    nc.vector.tensor_tensor(tph, gi, itr, op=AT.mult)
                    nc.vector.tensor_tensor(hi, hi, tph, op=AT.add)

                    # --- V1: conv rows ---
                    convSB = dp.tile([128, 1024], BF16, tag="convSB",
                                     name="convSB")
                    for gg in range(2):
                        psX = pp.tile([128, 512], F32, tag="V1", name="V1")
                        nc.tensor.matmul(psX[:], lhsT=cs("V1Ar")[0:F1, :],
                                         rhs=H[:, gg * 512:(gg + 1) * 512],
                                         start=True, stop=False)
                        nc.tensor.matmul(psX[:], lhsT=cs("nV1Ai")[0:F1, :],
                                         rhs=H[:, 1024 + gg * 512:
                                               1024 + (gg + 1) * 512],
                                         start=False, stop=True)
                        nc.scalar.copy(
                            convSB[:, gg * 512:(gg + 1) * 512], psX[:])

                    # --- scratch writes (Pool-issued so C loads aren't gated)
                    nc.gpsimd.dma_start(
                        scratch[r0:r0 + 8, 0:14336].rearrange(
                            "q (a b) -> q a b", a=112).transpose([1, 0, 2]),
                        convSB[0:112, :].rearrange("p (q b) -> p q b", b=128))
                    nc.gpsimd.dma_start(
                        scratch[r0:r0 + 8, 14336:14337].rearrange("q x -> x q"),
                        convSB[112:113, 0:1024:128])
                    nc.gpsimd.dma_start(
                        scratch[r0:r0 + 8, 14337:14593].rearrange(
                            "q (a b) -> q a b", a=2).transpose([1, 0, 2]),
                        convSB[0:2, :].rearrange("p (q b) -> p q b", b=128))

            # ---------------- C) 32K level ----------------
            with tc.tile_pool(name="p32", bufs=2) as dp:
                for cb in range(NB2):
                    r0 = cb * 4
                    D2c = dp.tile([57, 1024], BF16, tag="D2c", name="D2c")
                    tgtf = dp.tile([57, 1024], F32, tag="tgtf", name="tgtf")
                    D2t = dp.tile([57, 1024], BF16, tag="D2t", name="D2t")
                    nc.scalar.memzero(D2c[:])
                    nc.scalar.memzero(tgtf[:])
                    nc.sync.dma_start(
                        D2c[0:56, :].rearrange("p (q b) -> p q b", b=256),
                        scratch[r0:r0 + 4, 0:14336].rearrange(
                            "q (a b) -> q a b", a=56).transpose([1, 0, 2]))
                    nc.sync.dma_start(
                        D2c[56:57, 0:1024:256],
                        scratch[r0:r0 + 4, 14336:14337].rearrange("q x -> x q"))
                    nc.sync.dma_start(
                        tgtf[0:56, :].rearrange("p (q b) -> p q b", b=256),
                        target[r0:r0 + 4, 0:14336].rearrange(
                            "q (a b) -> q a b", a=56).transpose([1, 0, 2]))
                    nc.sync.dma_start(
                        tgtf[56:57, 0:1024:256],
                        target[r0:r0 + 4, 14336:14337].rearrange("q x -> x q"))
                    nc.scalar.copy(D2t[:], tgtf[:])

                    # --- s1: A2[t2half, (inp,c,q,ri,f1)] ---
                    A2 = dp.tile([128, 2080], BF16, tag="A32", name="A32")
                    c_eng = [nc.scalar, nc.vector, nc.scalar, nc.vector,
                             nc.scalar, nc.scalar, nc.scalar, nc.scalar]
                    ci = 0
                    for inp, D in ((0, D2c), (1, D2t)):
                        for c in range(2):
                            for qp in range(2):
                                ps = pp.tile([128, 260], F32, tag="s1",
                                             name="s1")
                                for h in range(2):
                                    q = qp * 2 + h
                                    nc.tensor.matmul(
                                        ps[:, h * 130:(h + 1) * 130],
                                        lhsT=D[:, q * 256 + c * 128:
                                               q * 256 + c * 128 + 128],
                                        rhs=cs("W1ri65")[0:57, :],
                                        start=True, stop=True)
                                dst0 = inp * 1040 + c * 520 + qp * 260
                                _cp(c_eng[ci % 8], A2[:, dst0:dst0 + 260], ps[:])
                                ci += 1

                    # --- fwd twiddle cmul (contig 520 runs, tw tiled) ---
                    B2 = dp.tile([128, 2080], BF16, tag="B32", name="B32")
                    tmpV = dp.tile([128, 1040], BF16, tag="tmpV2", name="tmpV2")
                    tmpP = dp.tile([128, 1040], BF16, tag="tmpP2", name="tmpP2")

                    def v32(t, lo, hi):
                        return t[:].rearrange("p (i x) -> p i x", i=2)[:, :, lo:hi]

                    twr = cs("twBtr").rearrange("p (a x) -> p a x", a=1)\
                        .to_broadcast([128, 2, 520])
                    twi = cs("twBti").rearrange("p (a x) -> p a x", a=1)\
                        .to_broadcast([128, 2, 520])
                    ar, ai = v32(A2, 0, 520), v32(A2, 520, 1040)
                    br, bi = v32(B2, 0, 520), v32(B2, 520, 1040)
                    tv = tmpV[:].rearrange("p (i x) -> p i x", i=2)
                    tp = tmpP[:].rearrange("p (i x) -> p i x", i=2)
                    nc.vector.tensor_tensor(br, ar, twr, op=AT.mult)
                    nc.vector.tensor_tensor(tv, ai, twi, op=AT.mult)
                    nc.vector.tensor_tensor(br, br, tv, op=AT.subtract)
                    nc.gpsimd.tensor_tensor(bi, ar, twi, op=AT.mult)
                    nc.vector.tensor_tensor(tp, ai, twr, op=AT.mult)
                    nc.vector.tensor_tensor(bi, bi, tp, op=AT.add)

                    # --- s2: Z2[f2, (inp,f2c,ri,260)] ---
                    Z2 = dp.tile([128, 2080], BF16, tag="Z32", name="Z32")
                    zi_ = 0
                    for inp in range(2):
                        for f2c in range(2):
                            pzr = pq.tile([128, 260], F32, tag="s2r", name="s2r")
                            pzi = pq.tile([128, 260], F32, tag="s2i", name="s2i")

                            def rv(c, ri):
                                base = inp * 1040 + ri * 520 + c * 260
                                return B2[:, base:base + 260]

                            for c in range(2):
                                nc.tensor.matmul(
                                    pzr[:], lhsT=cs(f"W2Br{c}{f2c}"),
                                    rhs=rv(c, 0), start=(c == 0), stop=False)
                                nc.tensor.matmul(
                                    pzr[:], lhsT=cs(f"nW2Bi{c}{f2c}"),
                                    rhs=rv(c, 1), start=False, stop=(c == 1))
                            for c in range(2):
                                nc.tensor.matmul(
                                    pzi[:], lhsT=cs(f"W2Bi{c}{f2c}"),
                                    rhs=rv(c, 0), start=(c == 0), stop=False)
                                nc.tensor.matmul(
                                    pzi[:], lhsT=cs(f"W2Br{c}{f2c}"),
                                    rhs=rv(c, 1), start=False, stop=(c == 1))
                            _cp(c_eng[zi_ % 8],
                                Z2[:, inp * 1040 + f2c * 260:
                                   inp * 1040 + f2c * 260 + 260], pzr[:])
                            _cp(c_eng[(zi_ + 1) % 8],
                                Z2[:, inp * 1040 + 520 + f2c * 260:
                                   inp * 1040 + 520 + f2c * 260 + 260], pzi[:])
                            zi_ += 2

                    # --- spectral: S2 = Zc * conj(Zt), contiguous 520 ---
                    S2 = dp.tile([128, 1040], BF16, tag="S32", name="S32")
                    zpr, zpi = Z2[:, 0:520], Z2[:, 520:1040]
                    zer, zei = Z2[:, 1040:1560], Z2[:, 1560:2080]
                    sr, si = S2[:, 0:520], S2[:, 520:1040]
                    tv2 = tmpV[:, 0:520]
                    tp2 = tmpP[:, 0:520]
                    nc.vector.tensor_tensor(sr, zpr, zer, op=AT.mult)
                    nc.vector.tensor_tensor(tv2, zpi, zei, op=AT.mult)
                    nc.vector.tensor_tensor(sr, sr, tv2, op=AT.add)
                    nc.gpsimd.tensor_tensor(si, zpi, zer, op=AT.mult)
                    nc.vector.tensor_tensor(tp2, zpr, zei, op=AT.mult)
                    nc.vector.tensor_tensor(si, si, tp2, op=AT.subtract)

                    # --- inverse G2: psum (Gr|Gi) per row ---
                    G2sb = dp.tile([F1, 2048], BF16, tag="G32", name="G32")
                    for q in range(4):
                        psG = pp.tile([F1, 512], F32, tag="G", name="G")
                        for f2c in range(2):
                            sro = f2c * 260 + q * 65
                            sio = 520 + f2c * 260 + q * 65
                            nc.tensor.matmul(
                                psG[:], lhsT=S2[:, sro:sro + 65],
                                rhs=cs(f"V2B_rI{f2c}"),
                                start=(f2c == 0), stop=False)
                            nc.tensor.matmul(
                                psG[:], lhsT=S2[:, sio:sio + 65],
                                rhs=cs(f"V2B_iR{f2c}"),
                                start=False, stop=(f2c == 1))
                        eng = nc.scalar if q % 2 == 0 else nc.vector
                        dstg = G2sb[:, 0:2048].rearrange(
                            "p (r x) -> p r x", r=2)[
                            :, :, q * 256:(q + 1) * 256]
                        _cp(eng, dstg,
                            psG[:].rearrange("p (r t) -> p r t", r=2))

                    # --- itw cmul: H2 = G2 * itwB (contiguous 1024) ---
                    H2 = dp.tile([F1, 2048], BF16, tag="H32", name="H32")
                    itr = cs("itwBr")[0:F1, :]
                    iti = cs("itwBi")[0:F1, :]
                    gr, gi = G2sb[:, 0:1024], G2sb[:, 1024:2048]
                    hr, hi = H2[:, 0:1024], H2[:, 1024:2048]
                    tvh = tmpV[:F1, 0:1024]
                    tph = tmpP[:F1, 0:1024]
                    nc.vector.tensor_tensor(hr, gr, itr, op=AT.mult)
                    nc.vector.tensor_tensor(tvh, gi, iti, op=AT.mult)
                    nc.vector.tensor_tensor(hr, hr, tvh, op=AT.subtract)
                    nc.gpsimd.tensor_tensor(hi, gr, iti, op=AT.mult)
                    nc.vector.tensor_tensor(tph, gi, itr, op=AT.mult)
                    nc.vector.tensor_tensor(hi, hi, tph, op=AT.add)

                    # --- V1B + mask/max + argmax candidates ---
                    ccm = dp.tile([128, 1024], BF16, tag="ccm", name="ccm")
                    for qp in range(2):
                        psX = pp.tile([128, 512], F32, tag="V1", name="V1")
                        nc.tensor.matmul(psX[:], lhsT=cs("V1Br")[0:F1, :],
                                         rhs=H2[:, qp * 512:(qp + 1) * 512],
                                         start=True, stop=False)
                        nc.tensor.matmul(psX[:], lhsT=cs("nV1Bi")[0:F1, :],
                                         rhs=H2[:, 1024 + qp * 512:
                                                1024 + (qp + 1) * 512],
                                         start=False, stop=True)
                        ccv = ccm[:, qp * 512:(qp + 1) * 512].rearrange(
                            "p (a x) -> p a x", a=2)
                        mb = cs("maskB").rearrange(
                            "p (a x) -> p a x", a=1).to_broadcast([128, 2, 256])
                        nc.vector.scalar_tensor_tensor(
                            ccv, psX[:].rearrange("p (a x) -> p a x", a=2),
                            1.0, mb, op0=AT.bypass, op1=AT.add)
                        nc.vector.tensor_reduce(
                            allmax[:, r0 + qp * 2:r0 + qp * 2 + 2], ccv,
                            axis=AX.X, op=AT.max)
                    # candidates for the 4 rows in one batch
                    eqm = dp.tile([128, 1024], BF16, tag="eqm", name="eqm")
                    selm = dp.tile([128, 1024], F32, tag="selm", name="selm")
                    ccv4 = ccm[:].rearrange("p (a x) -> p a x", a=4)
                    amb = allmax[:, r0:r0 + 4].rearrange(
                        "p (a x) -> p a x", x=1).to_broadcast([128, 4, 256])
                    nc.vector.tensor_tensor(
                        eqm[:].rearrange("p (a x) -> p a x", a=4),
                        ccv4, amb, op=AT.is_equal)
                    shb = cs("shvB").rearrange(
                        "p (a x) -> p a x", a=1).to_broadcast([128, 4, 256])
                    nc.gpsimd.tensor_tensor(
                        selm[:].rearrange("p (a x) -> p a x", a=4),
                        eqm[:].rearrange("p (a x) -> p a x", a=4),
                        shb, op=AT.mult)
                    nc.vector.tensor_reduce(
                        argshv[:, r0:r0 + 4],
                        selm[:].rearrange("p (a x) -> p a x", a=4),
                        axis=AX.X, op=AT.min)

            _ps1ctx.__exit__(None, None, None)
            _ps2ctx.__exit__(None, None, None)

            # ---------------- D) argmax tail -> shifts -> loss ----------------
            with (
                tc.tile_pool(name="amax", bufs=1) as dp,
                tc.tile_pool(name="psD", bufs=1, space="PSUM") as pd,
            ):
                ptA = pd.tile([R, 128], BF16, tag="ptA", name="ptA")
                nc.tensor.transpose(ptA[:], allmax[:, 0:R], cs("ident"))
                tmaxB = dp.tile([R, 128], BF16, tag="tmaxB", name="tmaxB")
                nc.scalar.copy(tmaxB[:], ptA[:])
                rowmax = dp.tile([R, 1], BF16, tag="rowmax", name="rowmax")
                nc.vector.tensor_reduce(rowmax[:], tmaxB[:], axis=AX.X,
                                        op=AT.max)
                prm = pd.tile([1, R], BF16, tag="prm", name="prm")
                nc.tensor.transpose(prm[:], rowmax[:], cs("ident")[0:R, 0:R])
                rmT = dp.tile([1, R], BF16, tag="rmT", name="rmT")
                nc.scalar.copy(rmT[:], prm[:])
                pmb = pd.tile([128, R], F32, tag="pmb", name="pmb")
                nc.tensor.matmul(pmb[:], lhsT=cs("W1r")[0:1, :],
                                 rhs=rmT[:], start=True, stop=True)
                MbB = dp.tile([128, R], BF16, tag="MbB", name="MbB")
                nc.scalar.copy(MbB[:], pmb[:])

                eq1 = dp.tile([128, R], BF16, tag="eq1", name="eq1")
                nc.vector.tensor_tensor(eq1[:], allmax[:, 0:R], MbB[:],
                                        op=AT.is_equal)
                selA = dp.tile([128, R], F32, tag="selA", name="selA")
                nc.vector.tensor_tensor(selA[:], eq1[:], argshv[:, 0:R],
                                        op=AT.mult)
                ptS = pd.tile([R, 128], F32, tag="ptS", name="ptS")
                nc.tensor.transpose(ptS[:], selA[:], cs("identF"))
                tminS = dp.tile([R, 128], F32, tag="tminS", name="tminS")
                nc.scalar.copy(tminS[:], ptS[:])
                nc.vector.tensor_reduce(shifts[:], tminS[:], axis=AX.X,
                                        op=AT.min)
                nc.vector.tensor_scalar_add(shifts[:], shifts[:],
                                            BIGL + float(START0))

                # start = (7040 + shift) mod 14337
                m1 = dp.tile([R, 1], F32, tag="m1", name="m1")
                nc.vector.tensor_scalar(out=m1[:], in0=shifts[:], scalar1=0.0,
                                        scalar2=None, op0=AT.is_lt)
                nc.vector.scalar_tensor_tensor(
                    shifts[:], m1[:], float(CONV_LEN), shifts[:],
                    op0=AT.mult, op1=AT.add)
                nc.vector.tensor_scalar(out=m1[:], in0=shifts[:],
                                        scalar1=float(CONV_LEN), scalar2=None,
                                        op0=AT.is_ge)
                nc.vector.scalar_tensor_tensor(
                    shifts[:], m1[:], float(-CONV_LEN), shifts[:],
                    op0=AT.mult, op1=AT.add)

                idxf = dp.tile([R, CROP], F32, tag="idxf", name="idxf")
                nc.vector.tensor_tensor(idxf[:], cs("winidx")[0:R, :],
                                        shifts[:].to_broadcast([R, CROP]),
                                        op=AT.add)
                idxi = dp.tile([R, CROP], I32, tag="idxi", name="idxi")
                nc.vector.tensor_copy(idxi[:], idxf[:])
                w = dp.tile([R, CROP], BF16, tag="wg", name="wg")
                nc.gpsimd.indirect_dma_start(
                    out=w[:], out_offset=None,
                    in_=scratch.ap().rearrange("r p -> (r p)").rearrange(
                        "(a b) -> a b", b=1),
                    in_offset=bass.IndirectOffsetOnAxis(ap=idxi[:], axis=0),
                )
                tw_ = dp.tile([R, CROP], F32, tag="twin", name="twin")
                nc.sync.dma_start(tw_[:], target[:, START0:START0 + CROP])
                nc.vector.tensor_tensor(w[:], w[:], tw_[:], op=AT.subtract)
                convacc = dp.tile([R, 1], F32, tag="convacc", name="convacc")
                nc.vector.scalar_tensor_tensor(
                    tw_[:], w[:], 1.0, w[:], op0=AT.bypass, op1=AT.mult,
                    accum_out=convacc[:])

                a0 = dp.tile([128, 1], F32, tag="a0", name="a0")
                nc.vector.tensor_reduce(a0[:], astf_acc[:], axis=AX.X,
                                        op=AT.add)
                psa = pd.tile([1, 1], F32, tag="psa", name="psa")
                nc.tensor.matmul(psa[:], lhsT=a0[:], rhs=cs("ones128"),
                                 start=True, stop=True)
                psc = pd.tile([1, 1], F32, tag="psc", name="psc")
                nc.tensor.matmul(psc[:], lhsT=convacc[:],
                                 rhs=cs("ones64")[0:R, :],
                                 start=True, stop=True)
                nc.scalar.copy(outt[:, 0:1], psa[:])
                nc.scalar.copy(outt[:, 1:2], psc[:])
                nc.sync.dma_start(out[:], outt[:])

    nc.finalize()
    return nc


_CACHE = {}


def get_built():
    if "nc" not in _CACHE:
        _CACHE["nc"] = build_nc()
        _CACHE["consts"] = make_packed_consts()[0]
    return _CACHE["nc"], _CACHE["consts"]


LAST_RESULT = {}


def kernel(pred_astf, true_astf, egf, target_waveform):
    import os
    from concourse.bass_utils import run_bass_kernel_spmd
    nc, consts = get_built()
    pred_astf = np.ascontiguousarray(np.asarray(pred_astf, np.float32))
    true_astf = np.ascontiguousarray(np.asarray(true_astf, np.float32))
    egf = np.ascontiguousarray(np.asarray(egf, np.float32))
    target_waveform = np.ascontiguousarray(
        np.asarray(target_waveform, np.float32))
    B = pred_astf.shape[0]
    per = B // NCORES
    in_maps = []
    for i in range(NCORES):
        sl = slice(i * per, (i + 1) * per)
        m = {"pred": pred_astf[sl], "true": true_astf[sl],
             "egf": egf[sl], "target": target_waveform[sl]}
        m.update(consts)
        in_maps.append(m)
    trace = os.environ.get("CONVALIGN_TRACE") == "1"
    res = run_bass_kernel_spmd(nc, in_maps, core_ids=list(range(NCORES)),
                               trace=trace)
    LAST_RESULT["res"] = res
    sums = np.stack([res.results[i]["out"][0] for i in range(NCORES)])
    loss_astf = np.float32(sums[:, 0].sum() / (B * L1))
    loss_conv = np.float32(sums[:, 1].sum() / (B * CROP))
    total = np.float32(loss_astf + loss_conv)
    return total, loss_astf, loss_conv
